# revision 25
# baseline (speedup 1.0000x reference)
"""Trainium2 Bass kernel for CausalWanSelfAttention (block-causal + local window + sink).

v2: static-instruction-count-minimized via hardware loops.

Strategy (8 NeuronCores, SPMD):
  - Sequence-sharded: core c owns tokens [384c, 384c+384).
  - One merged For_i(0,24) computes Q and K projections + RoPE (rotation
    tables host-precomputed, gains/scale folded in), storing rotated-but-
    unnormalized tiles to DRAM and per-tile square-sums to an SBUF strip.
  - RMS factors: two small fp32 matmuls reduce the strip, Rsqrt, broadcast.
  - K tiles normalized in a 12-iter loop -> AllGather; V projection in a
    3-iter loop -> AllGather; Q normalization deferred into attention.
  - Attention: For_i over 12 heads; context = sink frame (256) + 4-core
    window of the gathered K/V selected by runtime offsets; mask folded as
    additive bias into the exp activation; denominator via ones-matmul.
  - Output projection: 3-iter loop over token chunks.
"""
import os
import time
import numpy as np

import concourse.bass as bass
import concourse.tile as tile
from concourse import bacc, mybir
from concourse.bass_interp import get_hw_module

F32 = mybir.dt.float32
F32R = mybir.dt.float32r
BF16 = mybir.dt.bfloat16
AF = mybir.ActivationFunctionType
ALU = mybir.AluOpType

DIM = 1536
NH = 12
HD = 128
S = 3072
NC = 8
T = S // NC          # 384 tokens per core
NT = DIM // 128      # 12 o/i tiles
C = HD // 2          # 64 rope pairs
CTX = 1792           # 256 sink + 4*384 window
NCT = CTX // 128     # 14 ctx tiles
NFPB = 3
LOCAL = 6
SINK = 1
EPS = 1e-6
MASK_NEG = -1.0e4

# inputs that are identical on every core (replicated on the device mesh)
REPLICATED = {"wqk", "wv", "wo", "bqk", "swapm", "onesc", "onesrow", "selqk"}

_CACHE = {}
LAST_RESULT = None


def _emit(tc, repeat=1):
    nc = tc.nc
    from contextlib import ExitStack

    xt_d = nc.dram_tensor("xt", [DIM, T], BF16, kind="ExternalInput").ap()
    wqk_d = nc.dram_tensor("wqk", [DIM, 2 * DIM], BF16, kind="ExternalInput").ap()
    wv_d = nc.dram_tensor("wv", [DIM, DIM], BF16, kind="ExternalInput").ap()
    wo_d = nc.dram_tensor("wo", [DIM, DIM], BF16, kind="ExternalInput").ap()
    bqk_d = nc.dram_tensor("bqk", [128, 2 * NT], F32, kind="ExternalInput").ap()
    qkcos_d = nc.dram_tensor("qkcos", [2 * DIM, T], BF16, kind="ExternalInput").ap()
    qksin_d = nc.dram_tensor("qksin", [2 * DIM, T], BF16, kind="ExternalInput").ap()
    swap_d = nc.dram_tensor("swapm", [128, 128], BF16, kind="ExternalInput").ap()
    ones_d = nc.dram_tensor("onesc", [128, 1], BF16, kind="ExternalInput").ap()
    onesrow_d = nc.dram_tensor("onesrow", [1, 128], F32, kind="ExternalInput").ap()
    mask_d = nc.dram_tensor("maskb", [128, NCT], F32, kind="ExternalInput").ap()
    y_d = nc.dram_tensor("y", [T, DIM], F32, kind="ExternalOutput").ap()

    # scratch DRAM
    rot_d = nc.dram_tensor("rotd", [2 * NT * 128, T], BF16).ap()
    att_d = nc.dram_tensor("attd", [NT * 128, T], BF16).ap()
    sel_d = nc.dram_tensor("selqk", [2 * NT * 128, 33], BF16,
                           kind="ExternalInput").ap()
    agk_in = nc.dram_tensor("agk_in", [NT * 128, T], BF16).ap()
    agv_in = nc.dram_tensor("agv_in", [NT * 3 * 128, 128], BF16).ap()
    agk_out = nc.dram_tensor("agk_out", [NC * NT * 128, T], BF16,
                             addr_space="Shared").ap()
    agv_out = nc.dram_tensor("agv_out", [NC * NT * 3 * 128, 128], BF16,
                             addr_space="Shared").ap()

    with ExitStack() as top:
        persist = top.enter_context(tc.tile_pool(name="persist", bufs=1))
        swap_sb = persist.tile([128, 128], BF16, tag="swap")
        nc.sync.dma_start(swap_sb[:], swap_d)
        ones_sb = persist.tile([128, 1], BF16, tag="ones")
        nc.sync.dma_start(ones_sb[:], ones_d)
        ones12_sb = persist.tile([NT, 1], F32, tag="ones12")
        nc.vector.memset(ones12_sb[:], 1.0)
        onesrow_sb = persist.tile([1, 128], F32R, tag="onesrow")
        nc.sync.dma_start(onesrow_sb[:], onesrow_d.bitcast(F32R))
        mask_sb = persist.tile([128, NCT], F32, tag="mask")
        nc.sync.dma_start(mask_sb[:], mask_d)
        eps_sb = persist.tile([1, 1], F32, tag="eps")
        nc.vector.memset(eps_sb[:], EPS)
        zero_sb = persist.tile([1, 1], F32, tag="zero")
        nc.vector.memset(zero_sb[:], 0.0)

        # window base (in cores) from partition id: 2*(pid>=4) + 2*(pid>=6)
        e = nc.sync
        pid = e.partition_id()
        r1 = e.alloc_register("wge4")
        e.reg_alu(r1, pid, 3, ALU.subtract)
        e.reg_alu(r1, r1, 0, ALU.max)
        e.reg_alu(r1, r1, 1, ALU.min)
        r2 = e.alloc_register("wge6")
        e.reg_alu(r2, pid, 5, ALU.subtract)
        e.reg_alu(r2, r2, 0, ALU.max)
        e.reg_alu(r2, r2, 1, ALU.min)
        e.reg_alu(r1, r1, r2, ALU.add)
        e.reg_alu(r1, r1, 2, ALU.mult)
        w_sv = e.snap(r1, donate=True, min_val=0, max_val=4)

        zsel_sb = persist.tile([128, 33], BF16, tag="zsel")
        nc.vector.memset(zsel_sb[:], 0.0)

        for rep in range(repeat):
            _emit_once(tc, rep, w_sv, xt_d, wqk_d, wv_d, wo_d, bqk_d,
                       qkcos_d, qksin_d, y_d, rot_d, att_d, sel_d,
                       agk_in, agv_in, agk_out, agv_out,
                       swap_sb, ones_sb, ones12_sb, onesrow_sb, mask_sb,
                       eps_sb, zero_sb, zsel_sb)


def _emit_once(tc, rep, w_sv, xt_d, wqk_d, wv_d, wo_d, bqk_d,
               qkcos_d, qksin_d, y_d, rot_d, att_d, sel_d,
               agk_in, agv_in, agk_out, agv_out,
               swap_sb, ones_sb, ones12_sb, onesrow_sb, mask_sb,
               eps_sb, zero_sb, zsel_sb):
    nc = tc.nc
    from contextlib import ExitStack
    R = f"_r{rep}"

    wqkv = wqk_d.rearrange("(i p) c -> p i c", p=128)        # [128, 12, 3072]
    wvv = wv_d.rearrange("(i p) o -> p i o", p=128)          # [128, 12, 1536]
    wov = wo_d.rearrange("(i p) o -> p i o", p=128)
    qkcosv = qkcos_d.rearrange("(j p) t -> j p t", p=128)    # [24, 128, 384]
    qksinv = qksin_d.rearrange("(j p) t -> j p t", p=128)
    selv = sel_d.rearrange("(j p) s -> j p s", p=128)        # [24, 128, 2]
    rotv = rot_d.rearrange("(j p) t -> j p t", p=128)        # [24, 128, 384]
    attv = att_d.rearrange("(h p) t -> h p t", p=128)        # [12, 128, 384]
    agk_in_v = agk_in.rearrange("(h p) t -> h p t", p=128)   # [12, 128, 384]
    agv_in_v = agv_in.rearrange("(h b p) d -> h b p d", h=NT, b=3)
    kview = agk_out.rearrange("(r h p) t -> r h p t", r=NC, h=NT)
    vview = agv_out.rearrange("(r h b p) d -> r h b p d", r=NC, h=NT, b=3)

    with ExitStack() as body:
        xpool = body.enter_context(tc.tile_pool(name="xp" + R, bufs=1))
        fix = body.enter_context(tc.tile_pool(name="fix" + R, bufs=1))
        tmp = body.enter_context(tc.tile_pool(name="tmp" + R, bufs=2))

        xt_sb = xpool.tile([128, NT, T], BF16, tag="xt")
        nc.sync.dma_start(xt_sb[:], xt_d.rearrange("(n d) t -> d n t", n=NT))

        # ---- merged Q+K projection + rope: j in [0, 24) (q: 0-11, k: 12-23)
        wsl = fix.tile([128, NT, 128], BF16, tag="wsl")
        bias_f = fix.tile([128, 1], F32, tag="biasf")
        cos_f = fix.tile([128, T], BF16, tag="cosf")
        sin_f = fix.tile([128, T], BF16, tag="sinf")
        pre_f = fix.tile([128, T], BF16, tag="pref")
        sq_f = fix.tile([128, T], BF16, tag="sqf")
        m1_f = fix.tile([128, T], BF16, tag="m1f")
        m2_f = fix.tile([128, T], BF16, tag="m2f")
        rot_f = fix.tile([128, T], BF16, tag="rotf")
        sel_f = fix.tile([128, 33], BF16, tag="self")
        rd_qb = fix.tile([128, T], F32, tag="rdqb")

        pbk = body.enter_context(tc.tile_pool(name="pbk" + R, bufs=1, space="PSUM"))
        with ExitStack() as php:
            pmm = php.enter_context(tc.tile_pool(name="pmm" + R, bufs=1, space="PSUM"))
            pss = php.enter_context(tc.tile_pool(name="pss" + R, bufs=1, space="PSUM"))
            psw = php.enter_context(tc.tile_pool(name="psw" + R, bufs=1, space="PSUM"))
            ps = pmm.tile([128, T], F32, tag="ps")
            ps_qk = pss.tile([33, T], F32, tag="ssqk")
            ps_sw = psw.tile([128, T], F32, tag="swp")
            nc.vector.memset(ps_qk[:], 0.0)

            with tc.For_i(0, 2 * NT, 1) as j:
                nc.sync.dma_start(wsl[:], wqkv[:, :, bass.ds(j * 128, 128)])
                nc.scalar.dma_start(
                    cos_f[:].rearrange("p (j t) -> p j t", j=1),
                    qkcosv[bass.ds(j, 1), :, :].rearrange("j p t -> p j t"))
                nc.scalar.dma_start(
                    sin_f[:].rearrange("p (j t) -> p j t", j=1),
                    qksinv[bass.ds(j, 1), :, :].rearrange("j p t -> p j t"))
                nc.sync.dma_start(bias_f[:], bqk_d[:, bass.ds(j, 1)])
                nc.sync.dma_start(
                    sel_f[:].rearrange("p (j s) -> p j s", j=1),
                    selv[bass.ds(j, 1), :, :].rearrange("j p s -> p j s"))
                for i in range(NT):
                    nc.tensor.matmul(ps[:], wsl[:, i, :], xt_sb[:, i, :],
                                     start=(i == 0), stop=(i == NT - 1))
                nc.vector.tensor_scalar_add(pre_f[:], ps[:], bias_f[:])
                nc.vector.tensor_mul(sq_f[:], pre_f[:], pre_f[:])
                nc.tensor.matmul(ps_qk[:], sel_f[:], sq_f[:],
                                 start=False, stop=False, skip_group_check=True)
                nc.tensor.matmul(ps_sw[:], swap_sb[:], pre_f[:],
                                 start=True, stop=True)
                nc.vector.tensor_mul(m1_f[:], pre_f[:], cos_f[:])
                nc.vector.tensor_mul(m2_f[:], ps_sw[:], sin_f[:])
                nc.vector.tensor_add(rot_f[:], m1_f[:], m2_f[:])
                nc.sync.dma_start(
                    rotv[bass.ds(j, 1), :, :].rearrange("j p t -> p j t"),
                    rot_f[:].rearrange("p (j t) -> p j t", j=1))
            # end the open accumulation group (adds zeros)
            nc.tensor.matmul(ps_qk[:], zsel_sb[:], sq_f[:],
                             start=False, stop=True, skip_group_check=True)

            # ---- rms factors for q and k (k accumulator on partition 32)
            eps2_sb = fix.tile([33, 1], F32, tag="eps2")
            nc.vector.memset(eps2_sb[:], EPS)
            srt_qk = tmp.tile([33, T], F32, tag="srtqk")
            nc.scalar.activation(srt_qk[:], ps_qk[:], AF.Sqrt,
                                 bias=eps2_sb[:], scale=1.0 / DIM)
            rd_qk = tmp.tile([33, T], F32, tag="rdqk")
            nc.vector.reciprocal(rd_qk[:], srt_qk[:])
            rd_qr1 = tmp.tile([1, T], F32R, tag="rdqr1")
            nc.vector.tensor_copy(rd_qr1[:], rd_qk[0:1, :])
            rd_kr1 = tmp.tile([1, T], F32R, tag="rdkr1")
            nc.vector.tensor_copy(rd_kr1[:], rd_qk[32:33, :])
            ps_bq = psw.tile([128, T], F32, tag="bcq")
            nc.tensor.matmul(ps_bq[:], onesrow_sb[:], rd_qr1[:],
                             start=True, stop=True)
            nc.vector.tensor_copy(rd_qb[:], ps_bq[:])
            ps_bk = pbk.tile([128, T], F32, tag="bck")
            nc.tensor.matmul(ps_bk[:], onesrow_sb[:], rd_kr1[:],
                             start=True, stop=True)

            # ---- K normalize + export: j in [0, 12)
            rot2_f = fix.tile([128, T], BF16, tag="rot2f")
            kn_f = fix.tile([128, T], BF16, tag="knf")
            with tc.For_i(0, NT, 1) as j:
                nc.sync.dma_start(
                    rot2_f[:].rearrange("p (j t) -> p j t", j=1),
                    rotv[bass.ds(j + NT, 1), :, :].rearrange("j p t -> p j t"))
                nc.vector.tensor_mul(kn_f[:], rot2_f[:], ps_bk[:])
                nc.sync.dma_start(
                    agk_in_v[bass.ds(j, 1), :, :].rearrange("h p t -> p h t"),
                    kn_f[:].rearrange("p (h t) -> p h t", h=1))
        nc.gpsimd.collective_compute(
            "AllGather", mybir.AluOpType.bypass,
            ins=[agk_in], outs=[agk_out], replica_groups=[list(range(NC))])

        # ---- V projection: b in [0, 3) token chunks
        wv_sb = xpool.tile([128, NT, DIM], BF16, tag="wvsb")
        nc.sync.dma_start(wv_sb[:], wvv)
        xtb_f = fix.tile([128, NT, 128], BF16, tag="xtbf")
        with ExitStack() as phv:
            pv = phv.enter_context(tc.tile_pool(name="pv" + R, bufs=2, space="PSUM"))
            ps_v = [pv.tile([128, 512], F32, tag="psv", name=f"psv{_i}") for _i in range(2)]
            vsb = [tmp.tile([128, 512], BF16, tag="vsb", name=f"vsb{_i}") for _i in range(2)]
            with tc.For_i(0, 3, 1) as b:
                nc.sync.dma_start(xtb_f[:], xt_sb[:, :, bass.ds(b * 128, 128)])
                for oc in range(3):
                    psv = ps_v[oc % 2]
                    for i in range(NT):
                        nc.tensor.matmul(psv[:], xtb_f[:, i, :],
                                         wv_sb[:, i, 512 * oc:512 * (oc + 1)],
                                         start=(i == 0), stop=(i == NT - 1))
                    vs = vsb[oc % 2]
                    nc.vector.tensor_copy(vs[:], psv[:])
                    nc.sync.dma_start(
                        agv_in_v[4 * oc:4 * (oc + 1), bass.ds(b, 1), :, :]
                        .rearrange("h b p d -> p (h b) d"),
                        vs[:].rearrange("p (g d) -> p g d", g=4))
        nc.gpsimd.collective_compute(
            "AllGather", mybir.AluOpType.bypass,
            ins=[agv_in], outs=[agv_out], replica_groups=[list(range(NC))])

        # ---- attention: h in [0, 12)
        with ExitStack() as ph2:
            apool = ph2.enter_context(tc.tile_pool(name="attnp" + R, bufs=1))
            prp = ph2.enter_context(tc.tile_pool(name="probs" + R, bufs=3))
            ps_s_pool = ph2.enter_context(
                tc.tile_pool(name="pss2" + R, bufs=2, space="PSUM"))
            ps_o_pool = ph2.enter_context(
                tc.tile_pool(name="pso" + R, bufs=1, space="PSUM"))
            ps_d_pool = ph2.enter_context(
                tc.tile_pool(name="psd" + R, bufs=1, space="PSUM"))
            ps_b_pool = ph2.enter_context(
                tc.tile_pool(name="psb" + R, bufs=1, space="PSUM"))

            qh = apool.tile([128, T], BF16, tag="qh")
            qh_n = apool.tile([128, T], BF16, tag="qhn")
            kt = apool.tile([128, CTX], BF16, tag="kt")
            vt = apool.tile([128, NCT, 128], BF16, tag="vt")
            att_f = apool.tile([128, T], BF16, tag="attf")
            rd_a = apool.tile([1, T], F32, tag="rda")
            rd_ar = apool.tile([1, T], F32R, tag="rdar")
            rd_as = apool.tile([128, T], F32, tag="rdas")
            ps_s2 = [ps_s_pool.tile([128, T], F32, tag="s2", name=f"s2_{_i}") for _i in range(2)]
            ps_o = ps_o_pool.tile([128, T], F32, tag="o")
            ps_d = ps_d_pool.tile([1, T], F32, tag="d")
            ps_ab = ps_b_pool.tile([128, T], F32, tag="ab")
            prs = [prp.tile([128, T], BF16, tag="pr", name=f"pr{_i}") for _i in range(3)]

            with tc.For_i(0, NH, 1) as h:
                nc.sync.dma_start(
                    qh[:].rearrange("p (j t) -> p j t", j=1),
                    rotv[bass.ds(h, 1), :, :].rearrange("j p t -> p j t"))
                nc.vector.tensor_mul(qh_n[:], qh[:], rd_qb[:])
                nc.sync.dma_start(
                    kt[:, 0:256].rearrange("p (g t) -> p g t", g=1),
                    kview[0:1, bass.ds(h, 1), :, 0:256]
                    .rearrange("r h p t -> p (r h) t"))
                nc.sync.dma_start(
                    kt[:, 256:CTX].rearrange("p (g t) -> p g t", g=4),
                    kview[bass.ds(w_sv, 4), bass.ds(h, 1), :, :]
                    .rearrange("r h p t -> p (r h) t"))
                nc.sync.dma_start(
                    vt[:, 0:2, :],
                    vview[0:1, bass.ds(h, 1), 0:2, :, :]
                    .rearrange("r h b p d -> p (r h b) d"))
                for rr in range(4):
                    nc.sync.dma_start(
                        vt[:, 2 + 3 * rr:5 + 3 * rr, :],
                        vview[bass.ds(w_sv + rr, 1), bass.ds(h, 1), :, :, :]
                        .rearrange("r h b p d -> p (r h b) d"))
                for ct in range(NCT):
                    s2 = ps_s2[ct % 2]
                    nc.tensor.matmul(s2[:], kt[:, 128 * ct:128 * (ct + 1)],
                                     qh_n[:], start=True, stop=True)
                    pr = prs[ct % 3]
                    nc.scalar.activation(pr[:], s2[:], AF.Exp,
                                         bias=mask_sb[:, ct:ct + 1], scale=1.0)
                    nc.tensor.matmul(ps_o[:], vt[:, ct, :], pr[:],
                                     start=(ct == 0), stop=(ct == NCT - 1))
                    nc.tensor.matmul(ps_d[:], ones_sb[:], pr[:],
                                     start=(ct == 0), stop=(ct == NCT - 1))
                nc.vector.tensor_copy(rd_a[:], ps_d[:])
                nc.vector.tensor_copy(rd_ar[:], rd_a[:])
                nc.tensor.matmul(ps_ab[:], onesrow_sb[:], rd_ar[:],
                                 start=True, stop=True)
                nc.vector.reciprocal(rd_as[:], ps_ab[:])
                nc.vector.tensor_mul(att_f[:], ps_o[:], rd_as[:])
                nc.sync.dma_start(
                    attv[bass.ds(h, 1), :, :].rearrange("h p t -> p h t"),
                    att_f[:].rearrange("p (h t) -> p h t", h=1))

        # ---- output projection: b in [0, 3) token chunks
        wo_sb = xpool.tile([128, NT, DIM], BF16, tag="wosb")
        nc.sync.dma_start(wo_sb[:], wov)
        at_f = fix.tile([128, NT, 128], BF16, tag="atf")
        with ExitStack() as pho:
            po = pho.enter_context(tc.tile_pool(name="po" + R, bufs=2, space="PSUM"))
            ps_y = [po.tile([128, 512], F32, tag="psy", name=f"psy{_i}") for _i in range(2)]
            osb = [tmp.tile([128, 512], F32, tag="osb", name=f"osb{_i}") for _i in range(2)]
            with tc.For_i(0, 3, 1) as b:
                nc.sync.dma_start(
                    at_f[:],
                    attv[:, :, bass.ds(b * 128, 128)].rearrange("h p c -> p h c"))
                for oc in range(3):
                    psy = ps_y[oc % 2]
                    for i in range(NT):
                        nc.tensor.matmul(psy[:], at_f[:, i, :],
                                         wo_sb[:, i, 512 * oc:512 * (oc + 1)],
                                         start=(i == 0), stop=(i == NT - 1))
                    os_ = osb[oc % 2]
                    nc.vector.tensor_copy(os_[:], psy[:])
                    nc.sync.dma_start(
                        y_d[bass.ds(b * 128, 128), 512 * oc:512 * (oc + 1)],
                        os_[:])


def _build(repeat=1):
    key = ("nc", repeat)
    if key in _CACHE:
        return _CACHE[key]
    nc = bacc.Bacc("TRN2", target_bir_lowering=False, debug=False,
                   enable_asserts=False, num_devices=NC)
    with tile.TileContext(nc) as tc:
        _emit(tc, repeat)
    nc.compile()
    nc.m = get_hw_module(nc.m)
    _CACHE[key] = nc
    return nc


# ---------------------------------------------------------------------------
# host-side input preparation
# ---------------------------------------------------------------------------

def _pos_table(tab, f, h, w):
    cf = C - 2 * (C // 3)
    ch = C // 3
    tf = np.broadcast_to(tab[:f, :cf][:, None, None, :], (f, h, w, cf))
    th = np.broadcast_to(tab[:h, cf:cf + ch][None, :, None, :], (f, h, w, ch))
    tw = np.broadcast_to(tab[:w, cf + ch:][None, None, :, :], (f, h, w, ch))
    return np.concatenate([tf, th, tw], axis=-1).reshape(f * h * w, C)


def _rope_tables(cosP, sinP, g, scale):
    """(cosT, sinT) [S, DIM] folding g and the score scale.

    Device computes: rot = pre*cosT + swap(pre)*sinT, where swap exchanges
    even/odd partners. Equivalent to scale * rope(g * pre)."""
    cosE = np.repeat(cosP, 2, axis=1)          # [S, HD]
    sinE = np.repeat(sinP, 2, axis=1)
    cosT = np.empty((S, DIM), np.float32)
    sinT = np.empty((S, DIM), np.float32)
    for n in range(NH):
        gh = g[128 * n:128 * (n + 1)]
        cosT[:, 128 * n:128 * (n + 1)] = cosE * gh[None, :] * scale
        sh = np.empty(HD, np.float32)
        sh[0::2] = -gh[1::2]
        sh[1::2] = gh[0::2]
        sinT[:, 128 * n:128 * (n + 1)] = sinE * sh[None, :] * scale
    return cosT, sinT


def _mask_for_core(c):
    qb = c // 2
    frame = np.arange(S) // 256
    blk = frame // NFPB

    def allowed(k):
        return (blk[k] <= qb) & (((qb - blk[k]) * NFPB < LOCAL) | (frame[k] < SINK))

    m = np.full(CTX, MASK_NEG, np.float32)
    if qb >= 2:
        m[0:256] = 0.0
    wbase = 2 * max(qb - 1, 0)
    tok = np.arange(T * wbase, T * wbase + 1536)
    m[256:] = np.where(allowed(tok), 0.0, MASK_NEG)
    return np.ascontiguousarray(m.reshape(NCT, 128).T)  # [128, NCT]


def _prep_in_maps(x, Wq, bq, Wk, bk, Wv, bv, Wo, bo, gq, gk, freqs_cos, freqs_sin, f, h, w):
    x = np.asarray(x, np.float32)
    f, h, w = int(f), int(h), int(w)
    cosP = _pos_table(np.asarray(freqs_cos, np.float32), f, h, w)
    sinP = _pos_table(np.asarray(freqs_sin, np.float32), f, h, w)

    qcosT, qsinT = _rope_tables(cosP, sinP, np.asarray(gq, np.float32), HD ** -0.5)
    kcosT, ksinT = _rope_tables(cosP, sinP, np.asarray(gk, np.float32), 1.0)

    import ml_dtypes
    BF = ml_dtypes.bfloat16
    wq_t = np.ascontiguousarray(np.asarray(Wq, np.float32).T)
    wk_t = np.ascontiguousarray(np.asarray(Wk, np.float32).T)
    wqk_t = np.concatenate([wq_t, wk_t], axis=1).astype(BF)      # [DIM, 2*DIM]
    wv_t = np.ascontiguousarray(np.asarray(Wv, np.float32).T).astype(BF)
    wo_t = np.ascontiguousarray(np.asarray(Wo, np.float32).T).astype(BF)
    bq_r = np.asarray(bq, np.float32).reshape(NT, 128).T
    bk_r = np.asarray(bk, np.float32).reshape(NT, 128).T
    bqk_r = np.ascontiguousarray(np.concatenate([bq_r, bk_r], axis=1))  # [128, 24]

    swapm = np.zeros((128, 128), np.float32)
    idx = np.arange(128)
    swapm[idx, idx ^ 1] = 1.0
    swapm = swapm.astype(BF)
    onesc = np.ones((128, 1), BF)
    onesrow = np.ones((1, 128), np.float32)
    sel = np.zeros((2 * NT * 128, 33), np.float32)
    sel[0:NT * 128, 0] = 1.0
    sel[NT * 128:, 32] = 1.0
    sel = sel.astype(BF)

    xs = x[0]  # [S, DIM]
    in_maps = []
    for c in range(NC):
        xt_c = np.ascontiguousarray(xs[T * c:T * (c + 1), :].T).astype(BF)
        sl = slice(T * c, T * (c + 1))
        qkcos = np.concatenate([qcosT[sl].T, kcosT[sl].T], axis=0)  # [2*DIM, T]
        qksin = np.concatenate([qsinT[sl].T, ksinT[sl].T], axis=0)
        in_maps.append(dict(
            xt=xt_c, wqk=wqk_t, wv=wv_t, wo=wo_t, bqk=bqk_r,
            qkcos=np.ascontiguousarray(qkcos).astype(BF),
            qksin=np.ascontiguousarray(qksin).astype(BF),
            swapm=swapm, onesc=onesc, onesrow=onesrow, selqk=sel,
            maskb=_mask_for_core(c),
        ))

    bo_eff = np.asarray(bo, np.float32) + np.asarray(bv, np.float32) @ np.asarray(Wo, np.float32).T
    return in_maps, bo_eff


def _assemble(per_core_y, bo_eff):
    out = np.concatenate(per_core_y, axis=0)  # [S, DIM]
    out = out + bo_eff[None, :]
    return out[None].astype(np.float32)


# ---------------------------------------------------------------------------
# execution (PJRT shard_map; replicated specs for weights)
# ---------------------------------------------------------------------------

def _make_runner(nc):
    import jax
    from jax.sharding import Mesh, PartitionSpec
    try:
        from jax.experimental.shard_map import shard_map
    except ImportError:
        from jax.shard_map import shard_map
    from concourse.bass2jax import _bass_exec_p, install_neuronx_cc_hook, partition_id_tensor

    install_neuronx_cc_hook()
    partition_name = nc.partition_id_tensor.name if nc.partition_id_tensor else None
    in_names, out_names, out_avals = [], [], []
    for alloc in nc.m.functions[0].allocations:
        if not isinstance(alloc, mybir.MemoryLocationSet):
            continue
        name = alloc.memorylocations[0].name
        if alloc.kind == "ExternalInput":
            if name != partition_name:
                in_names.append(name)
        elif alloc.kind == "ExternalOutput":
            out_names.append(name)
            out_avals.append(jax.core.ShapedArray(tuple(alloc.tensor_shape),
                                                  mybir.dt.np(alloc.dtype)))
    n_params = len(in_names)
    all_in_names = list(in_names) + out_names
    if partition_name is not None:
        all_in_names.append(partition_name)

    def _body(*args):
        ins = list(args[:n_params])
        zouts = list(args[n_params:])
        extra = [partition_id_tensor()] if partition_name is not None else []
        outs = _bass_exec_p.bind(
            *ins, *zouts, *extra,
            out_avals=tuple(out_avals),
            in_names=tuple(all_in_names),
            out_names=tuple(out_names),
            lowering_input_output_aliases=(),
            sim_require_finite=False,
            sim_require_nnan=False,
            nc=nc,
        )
        return tuple(outs)

    import numpy as _np
    devices = jax.devices()[:NC]
    mesh = Mesh(_np.asarray(devices), ("core",))
    in_specs = tuple(
        PartitionSpec() if name in REPLICATED else PartitionSpec("core")
        for name in in_names
    ) + (PartitionSpec("core"),) * len(out_names)
    out_specs = (PartitionSpec("core"),) * len(out_names)
    fn = jax.jit(shard_map(_body, mesh=mesh, in_specs=in_specs,
                           out_specs=out_specs, check_rep=False))
    return fn, in_names, out_names, out_avals


def _prepare_args(in_maps, in_names, out_avals):
    import jax
    args = []
    for i, name in enumerate(in_names):
        if name in REPLICATED:
            args.append(in_maps[0][name])
        else:
            args.append(np.concatenate([np.asarray(m[name]) for m in in_maps], axis=0))
    for a in out_avals:
        args.append(np.zeros((NC * a.shape[0], *a.shape[1:]), a.dtype))
    return [jax.device_put(a) for a in args]


def _run(nc, in_maps):
    import jax
    key = ("runner", id(nc))
    if key not in _CACHE:
        _CACHE[key] = _make_runner(nc)
    fn, in_names, out_names, out_avals = _CACHE[key]
    args = _prepare_args(in_maps, in_names, out_avals)
    outs = fn(*args)
    jax.block_until_ready(outs)
    results = []
    for c in range(NC):
        r = {}
        for i, name in enumerate(out_names):
            r[name] = np.asarray(outs[i]).reshape(NC, *out_avals[i].shape)[c]
        results.append(r)
    return results


def kernel(**inputs):
    global LAST_RESULT
    in_maps, bo_eff = _prep_in_maps(**inputs)
    nc = _build()
    results = _run(nc, in_maps)
    LAST_RESULT = results
    return _assemble([results[c]["y"] for c in range(NC)], bo_eff)


# revision 32
# speedup vs baseline: 1.1884x; 1.1884x over previous
"""Trainium2 Bass kernel for CausalWanSelfAttention (block-causal + local window + sink).

v3: ZERO-COLLECTIVE design. The per-rep cost on this runtime is dominated by
collective ops / cross-core sync, not instruction count, so each core
redundantly computes the K/V projections for its own attention context
instead of exchanging K/V.

Strategy (8 NeuronCores, SPMD, fully unrolled):
  - Sequence-sharded queries: core c owns tokens [384c, 384c+384).
  - Host hands each core x^T for its 1792-token context (sink frame 256 +
    4-core window 1536) alongside x^T for its own 384 tokens.
  - Each core computes: Q projection (own tokens) + K,V projections (its
    full context), RMS-norm + RoPE via host-precomputed tables (gains and
    the 1/sqrt(hd) score scale folded in), entirely in SBUF.
  - Attention: per head, dense [ctx=1792] context; invalid/duplicate
    context suppressed by an additive -1e4 bias fused into the exp.
    Softmax denominator via a ones-column matmul; normalization applied to
    the attention output via broadcast + reciprocal.
  - Output projection from SBUF-resident attention output. No collectives,
    no DRAM scratch, no inter-core traffic of any kind.
"""
import os
import time
import numpy as np

import concourse.bass as bass
import concourse.tile as tile
from concourse import bacc, mybir
from concourse.bass_interp import get_hw_module

F32 = mybir.dt.float32
F32R = mybir.dt.float32r
BF16 = mybir.dt.bfloat16
AF = mybir.ActivationFunctionType
ALU = mybir.AluOpType

DIM = 1536
NH = 12
HD = 128
S = 3072
NC = 8
T = S // NC          # 384 tokens per core
NT = DIM // 128      # 12 o/i tiles
C = HD // 2          # 64 rope pairs
CTX = 1792           # 256 sink + 4*384 window
NCT = CTX // 128     # 14 ctx tiles (tokens-on-partition chunks)
NKC = 4              # K-projection free-dim chunks of 448
KCW = CTX // NKC     # 448
NFPB = 3
LOCAL = 6
SINK = 1
EPS = 1e-6
MASK_NEG = -1.0e4

REPLICATED = {"wq", "wk", "wv", "wo", "bqr", "bkr", "swapm", "onesc", "onesrow"}

_CACHE = {}
LAST_RESULT = None


def _emit(tc, repeat=1):
    nc = tc.nc

    xq_d = nc.dram_tensor("xq", [DIM, T], BF16, kind="ExternalInput").ap()
    xc_d = nc.dram_tensor("xc", [DIM, CTX], BF16, kind="ExternalInput").ap()
    wq_d = nc.dram_tensor("wq", [DIM, DIM], BF16, kind="ExternalInput").ap()
    wk_d = nc.dram_tensor("wk", [DIM, DIM], BF16, kind="ExternalInput").ap()
    wv_d = nc.dram_tensor("wv", [DIM, DIM], BF16, kind="ExternalInput").ap()
    wo_d = nc.dram_tensor("wo", [DIM, DIM], BF16, kind="ExternalInput").ap()
    bq_d = nc.dram_tensor("bqr", [128, NT], F32, kind="ExternalInput").ap()
    bk_d = nc.dram_tensor("bkr", [128, NT], F32, kind="ExternalInput").ap()
    qcos_d = nc.dram_tensor("qcos", [DIM, T], BF16, kind="ExternalInput").ap()
    qsin_d = nc.dram_tensor("qsin", [DIM, T], BF16, kind="ExternalInput").ap()
    kcos_d = nc.dram_tensor("kcos", [DIM, CTX], BF16, kind="ExternalInput").ap()
    ksin_d = nc.dram_tensor("ksin", [DIM, CTX], BF16, kind="ExternalInput").ap()
    swap_d = nc.dram_tensor("swapm", [128, 128], BF16, kind="ExternalInput").ap()
    ones_d = nc.dram_tensor("onesc", [128, 1], BF16, kind="ExternalInput").ap()
    onesrow_d = nc.dram_tensor("onesrow", [1, 128], F32, kind="ExternalInput").ap()
    mask_d = nc.dram_tensor("maskb", [128, NCT], F32, kind="ExternalInput").ap()
    y_d = nc.dram_tensor("y", [T, DIM], F32, kind="ExternalOutput").ap()

    from contextlib import ExitStack
    with ExitStack() as top:
        persist = top.enter_context(tc.tile_pool(name="persist", bufs=1))
        swap_sb = persist.tile([128, 128], BF16, tag="swap")
        nc.sync.dma_start(swap_sb[:], swap_d)
        ones_sb = persist.tile([128, 1], BF16, tag="ones")
        nc.sync.dma_start(ones_sb[:], ones_d)
        onesrow_sb = persist.tile([1, 128], F32R, tag="onesrow")
        nc.sync.dma_start(onesrow_sb[:], onesrow_d.bitcast(F32R))
        mask_sb = persist.tile([128, NCT], F32, tag="mask")
        nc.sync.dma_start(mask_sb[:], mask_d)
        bq_sb = persist.tile([128, NT], F32, tag="bq")
        nc.sync.dma_start(bq_sb[:], bq_d)
        bk_sb = persist.tile([128, NT], F32, tag="bk")
        nc.sync.dma_start(bk_sb[:], bk_d)
        eps_sb = persist.tile([1, 1], F32, tag="eps")
        nc.vector.memset(eps_sb[:], EPS)

        for rep in range(repeat):
            _emit_once(tc, rep, xq_d, xc_d, wq_d, wk_d, wv_d, wo_d,
                       qcos_d, qsin_d, kcos_d, ksin_d, y_d,
                       swap_sb, ones_sb, onesrow_sb, mask_sb,
                       bq_sb, bk_sb, eps_sb)


def _emit_once(tc, rep, xq_d, xc_d, wq_d, wk_d, wv_d, wo_d,
               qcos_d, qsin_d, kcos_d, ksin_d, y_d,
               swap_sb, ones_sb, onesrow_sb, mask_sb, bq_sb, bk_sb, eps_sb):
    nc = tc.nc
    from contextlib import ExitStack
    R = f"_r{rep}"

    with ExitStack() as body:
        big = body.enter_context(tc.tile_pool(name="big" + R, bufs=1))
        tmp = body.enter_context(tc.tile_pool(name="tmp" + R, bufs=2))

        k_ctx = big.tile([128, NT, CTX], BF16, tag="kctx")
        v_ctx = big.tile([128, NCT, NT * 128], BF16, tag="vctx")
        q_rot = big.tile([128, NT, T], BF16, tag="qrot")
        attnT = big.tile([128, NT, T], BF16, tag="attnT")
        rd_qb = big.tile([128, T], F32, tag="rdqb")

        with ExitStack() as phkv:
            xcp = phkv.enter_context(tc.tile_pool(name="xcp" + R, bufs=1))
            tmpk = phkv.enter_context(tc.tile_pool(name="tmpk" + R, bufs=2))

            HCT = CTX // 2   # 896 ctx tokens per half
            xcv = xc_d.rearrange("(n d) t -> d n t", n=NT)
            wkv = wk_d.rearrange("(i p) o -> p i o", p=128)

            # ======= K projection over the context (rope fused, norm deferred)
            with ExitStack() as phk:
                wkp = phk.enter_context(tc.tile_pool(name="wkp" + R, bufs=1))
                tabp = phk.enter_context(tc.tile_pool(name="tabp" + R, bufs=2))
                pk = phk.enter_context(tc.tile_pool(name="pk" + R, bufs=2, space="PSUM"))
                pks = phk.enter_context(tc.tile_pool(name="pks" + R, bufs=1, space="PSUM"))
                pkw = phk.enter_context(tc.tile_pool(name="pkw" + R, bufs=1, space="PSUM"))
                pkb = phk.enter_context(tc.tile_pool(name="pkb" + R, bufs=1, space="PSUM"))
                kss_t = [pks.tile([1, KCW], F32, tag=f"kss{_i}", name=f"kss{_i}" + R)
                         for _i in range(NKC)]
                for wh in range(2):
                    wk_sb = wkp.tile([128, NT, 768], BF16, tag="w",
                                     name=f"wk{wh}" + R)
                    nc.sync.dma_start(wk_sb[:], wkv[:, :, 768 * wh:768 * (wh + 1)])
                    for ch in range(2):
                        xc_sb = xcp.tile([128, NT, HCT], BF16, tag="xc",
                                         name=f"xck{wh}_{ch}" + R)
                        nc.sync.dma_start(xc_sb[:], xcv[:, :, HCT * ch:HCT * (ch + 1)])
                        for n in range(6 * wh, 6 * (wh + 1)):
                            nw = n - 6 * wh
                            kcos_n = tabp.tile([128, HCT], BF16, tag="kcos",
                                               name=f"kc{n}_{ch}" + R)
                            nc.scalar.dma_start(
                                kcos_n[:],
                                kcos_d[128 * n:128 * (n + 1), HCT * ch:HCT * (ch + 1)])
                            ksin_n = tabp.tile([128, HCT], BF16, tag="ksin",
                                               name=f"ks{n}_{ch}" + R)
                            nc.scalar.dma_start(
                                ksin_n[:],
                                ksin_d[128 * n:128 * (n + 1), HCT * ch:HCT * (ch + 1)])
                            for c4 in range(2):
                                cc = 2 * ch + c4
                                sl = slice(HCT * ch + KCW * c4, HCT * ch + KCW * (c4 + 1))
                                slh = slice(KCW * c4, KCW * (c4 + 1))
                                ps_k = pk.tile([128, KCW], F32, tag="psk",
                                               name=f"psk{n}_{cc}" + R)
                                for i in range(NT):
                                    nc.tensor.matmul(
                                        ps_k[:], wk_sb[:, i, 128 * nw:128 * (nw + 1)],
                                        xc_sb[:, i, slh],
                                        start=(i == 0), stop=(i == NT - 1))
                                pre_f = tmpk.tile([128, KCW], BF16, tag="prek",
                                                  name=f"prek{n}_{cc}" + R)
                                nc.vector.tensor_scalar_add(pre_f[:], ps_k[:],
                                                            bk_sb[:, n:n + 1])
                                sq_f = tmpk.tile([128, KCW], BF16, tag="sqk",
                                                 name=f"sqk{n}_{cc}" + R)
                                nc.vector.tensor_mul(sq_f[:], pre_f[:], pre_f[:])
                                nc.tensor.matmul(kss_t[cc][:], ones_sb[:], sq_f[:],
                                                 start=(n == 0), stop=(n == NT - 1))
                                ps_sw = pkw.tile([128, KCW], F32, tag="swk",
                                                 name=f"swk{n}_{cc}" + R)
                                nc.tensor.matmul(ps_sw[:], swap_sb[:], pre_f[:],
                                                 start=True, stop=True)
                                m1_f = tmpk.tile([128, KCW], BF16, tag="m1k",
                                                 name=f"m1k{n}_{cc}" + R)
                                nc.vector.tensor_mul(m1_f[:], pre_f[:], kcos_n[:, slh])
                                m2_f = tmpk.tile([128, KCW], BF16, tag="m2k",
                                                 name=f"m2k{n}_{cc}" + R)
                                nc.vector.tensor_mul(m2_f[:], ps_sw[:], ksin_n[:, slh])
                                nc.vector.tensor_add(k_ctx[:, n, sl], m1_f[:], m2_f[:])

                # rms factors for k over the context, normalize k_ctx in place
                for cc in range(NKC):
                    sl = slice(KCW * cc, KCW * (cc + 1))
                    srt_k = tmpk.tile([1, KCW], F32, tag="srtk", bufs=1,
                                      name=f"srtk{cc}" + R)
                    nc.scalar.activation(srt_k[:], kss_t[cc][:],
                                         AF.Sqrt, bias=eps_sb[:], scale=1.0 / DIM)
                    rd_k = tmpk.tile([1, KCW], F32, tag="rdk", bufs=1,
                                     name=f"rdk{cc}" + R)
                    nc.vector.reciprocal(rd_k[:], srt_k[:])
                    rd_kr = tmpk.tile([1, KCW], F32R, tag="rdkr", bufs=1,
                                      name=f"rdkr{cc}" + R)
                    nc.vector.tensor_copy(rd_kr[:], rd_k[:])
                    ps_kb = pkb.tile([128, KCW], F32, tag="kb", name=f"kb{cc}" + R)
                    nc.tensor.matmul(ps_kb[:], onesrow_sb[:], rd_kr[:],
                                     start=True, stop=True)
                    for n in range(NT):
                        nc.vector.tensor_mul(k_ctx[:, n, sl], k_ctx[:, n, sl],
                                             ps_kb[:])

            # ======= V projection over the context (token-major)
            with ExitStack() as phv:
                wvp = phv.enter_context(tc.tile_pool(name="wvp" + R, bufs=1))
                pv = phv.enter_context(tc.tile_pool(name="pv" + R, bufs=2, space="PSUM"))
                wv_sb = wvp.tile([128, NT, DIM], BF16, tag="wv", name="wv" + R)
                nc.sync.dma_start(wv_sb[:],
                                  wv_d.rearrange("(i p) o -> p i o", p=128))
                for ch in range(2):
                    xc_sb = xcp.tile([128, NT, HCT], BF16, tag="xc",
                                     name=f"xcv{ch}" + R)
                    nc.sync.dma_start(xc_sb[:], xcv[:, :, HCT * ch:HCT * (ch + 1)])
                    for cth in range(NCT // 2):
                        ct = ch * (NCT // 2) + cth
                        for oc in range(3):
                            ps_v = pv.tile([128, 512], F32, tag="psv",
                                           name=f"psv{ct}_{oc}" + R)
                            for i in range(NT):
                                nc.tensor.matmul(
                                    ps_v[:], xc_sb[:, i, 128 * cth:128 * (cth + 1)],
                                    wv_sb[:, i, 512 * oc:512 * (oc + 1)],
                                    start=(i == 0), stop=(i == NT - 1))
                            nc.vector.tensor_copy(
                                v_ctx[:, ct, 512 * oc:512 * (oc + 1)], ps_v[:])

        # =========== Q projection (own tokens) + rope; norm deferred to attn
        with ExitStack() as phq:
            qp = phq.enter_context(tc.tile_pool(name="qp" + R, bufs=1))
            xq_sb = qp.tile([128, NT, T], BF16, tag="xq")
            nc.sync.dma_start(xq_sb[:], xq_d.rearrange("(n d) t -> d n t", n=NT))
            wq_sb = qp.tile([128, NT, DIM], BF16, tag="wq", name="wq" + R)
            nc.sync.dma_start(wq_sb[:], wq_d.rearrange("(i p) o -> p i o", p=128))
            qcos_sb = qp.tile([128, NT, T], BF16, tag="qcos", name="qcos" + R)
            nc.scalar.dma_start(qcos_sb[:], qcos_d.rearrange("(n d) t -> d n t", n=NT))
            qsin_sb = qp.tile([128, NT, T], BF16, tag="qsin", name="qsin" + R)
            nc.scalar.dma_start(qsin_sb[:], qsin_d.rearrange("(n d) t -> d n t", n=NT))
            pq = phq.enter_context(tc.tile_pool(name="pq" + R, bufs=2, space="PSUM"))
            pqs = phq.enter_context(tc.tile_pool(name="pqs" + R, bufs=1, space="PSUM"))
            pqw = phq.enter_context(tc.tile_pool(name="pqw" + R, bufs=2, space="PSUM"))
            pqb = phq.enter_context(tc.tile_pool(name="pqb" + R, bufs=1, space="PSUM"))
            qss = pqs.tile([1, T], F32, tag="qss")
            for n in range(NT):
                ps_q = pq.tile([128, T], F32, tag="psq", name=f"psq{n}" + R)
                for i in range(NT):
                    nc.tensor.matmul(ps_q[:], wq_sb[:, i, 128 * n:128 * (n + 1)],
                                     xq_sb[:, i, :],
                                     start=(i == 0), stop=(i == NT - 1))
                pre_f = tmp.tile([128, T], BF16, tag="preq", name=f"preq{n}" + R)
                nc.vector.tensor_scalar_add(pre_f[:], ps_q[:], bq_sb[:, n:n + 1])
                sq_f = tmp.tile([128, T], BF16, tag="sqq", name=f"sqq{n}" + R)
                nc.vector.tensor_mul(sq_f[:], pre_f[:], pre_f[:])
                nc.tensor.matmul(qss[:], ones_sb[:], sq_f[:],
                                 start=(n == 0), stop=(n == NT - 1))
                ps_sw = pqw.tile([128, T], F32, tag="swq", name=f"swq{n}" + R)
                nc.tensor.matmul(ps_sw[:], swap_sb[:], pre_f[:],
                                 start=True, stop=True)
                m1_f = tmp.tile([128, T], BF16, tag="m1q", name=f"m1q{n}" + R)
                nc.vector.tensor_mul(m1_f[:], pre_f[:], qcos_sb[:, n, :])
                m2_f = tmp.tile([128, T], BF16, tag="m2q", name=f"m2q{n}" + R)
                nc.vector.tensor_mul(m2_f[:], ps_sw[:], qsin_sb[:, n, :])
                nc.vector.tensor_add(q_rot[:, n, :], m1_f[:], m2_f[:])
            srt_q = tmp.tile([1, T], F32, tag="srtq")
            nc.scalar.activation(srt_q[:], qss[:], AF.Sqrt,
                                 bias=eps_sb[:], scale=1.0 / DIM)
            rd_q = tmp.tile([1, T], F32, tag="rdq")
            nc.vector.reciprocal(rd_q[:], srt_q[:])
            rd_qr = tmp.tile([1, T], F32R, tag="rdqr")
            nc.vector.tensor_copy(rd_qr[:], rd_q[:])
            ps_qb = pqb.tile([128, T], F32, tag="qb")
            nc.tensor.matmul(ps_qb[:], onesrow_sb[:], rd_qr[:],
                             start=True, stop=True)
            nc.vector.tensor_copy(rd_qb[:], ps_qb[:])

        # =========== attention (all operands SBUF-resident)
        with ExitStack() as pha:
            apool = pha.enter_context(tc.tile_pool(name="ap" + R, bufs=2))
            prp = pha.enter_context(tc.tile_pool(name="prp" + R, bufs=3))
            psS = pha.enter_context(tc.tile_pool(name="psS" + R, bufs=2, space="PSUM"))
            psO = pha.enter_context(tc.tile_pool(name="psO" + R, bufs=2, space="PSUM"))
            psD = pha.enter_context(tc.tile_pool(name="psD" + R, bufs=2, space="PSUM"))
            psB = pha.enter_context(tc.tile_pool(name="psB" + R, bufs=2, space="PSUM"))

            for h in range(NH):
                qh_n = apool.tile([128, T], BF16, tag="qhn", name=f"qhn{h}" + R)
                nc.vector.tensor_mul(qh_n[:], q_rot[:, h, :], rd_qb[:])
                ps_o = psO.tile([128, T], F32, tag="o", name=f"o{h}" + R)
                ps_d = psD.tile([1, T], F32, tag="d", name=f"d{h}" + R)
                for ct in range(NCT):
                    ps_s = psS.tile([128, T], F32, tag="s", name=f"s{h}_{ct}" + R)
                    nc.tensor.matmul(ps_s[:], k_ctx[:, h, 128 * ct:128 * (ct + 1)],
                                     qh_n[:], start=True, stop=True)
                    pr = prp.tile([128, T], BF16, tag="pr", name=f"pr{h}_{ct}" + R)
                    nc.scalar.activation(pr[:], ps_s[:], AF.Exp,
                                         bias=mask_sb[:, ct:ct + 1], scale=1.0)
                    nc.tensor.matmul(ps_o[:], v_ctx[:, ct, 128 * h:128 * (h + 1)], pr[:],
                                     start=(ct == 0), stop=(ct == NCT - 1))
                    nc.tensor.matmul(ps_d[:], ones_sb[:], pr[:],
                                     start=(ct == 0), stop=(ct == NCT - 1))
                d_f = tmp.tile([1, T], F32, tag="df", name=f"df{h}" + R)
                nc.vector.tensor_copy(d_f[:], ps_d[:])
                d_r = tmp.tile([1, T], F32R, tag="dr", name=f"dr{h}" + R)
                nc.vector.tensor_copy(d_r[:], d_f[:])
                ps_b = psB.tile([128, T], F32, tag="b", name=f"b{h}" + R)
                nc.tensor.matmul(ps_b[:], onesrow_sb[:], d_r[:],
                                 start=True, stop=True)
                rd_a = apool.tile([128, T], F32, tag="rda", name=f"rda{h}" + R)
                nc.vector.reciprocal(rd_a[:], ps_b[:])
                nc.vector.tensor_mul(attnT[:, h, :], ps_o[:], rd_a[:])

        # =========== output projection
        with ExitStack() as pho:
            wop = pho.enter_context(tc.tile_pool(name="wop" + R, bufs=1))
            wo_sb = wop.tile([128, NT, DIM], BF16, tag="wo", name="wo" + R)
            nc.sync.dma_start(wo_sb[:], wo_d.rearrange("(i p) o -> p i o", p=128))
            py = pho.enter_context(tc.tile_pool(name="py" + R, bufs=2, space="PSUM"))
            for tc_i in range(3):
                for oc in range(3):
                    ps_y = py.tile([128, 512], F32, tag="psy", name=f"psy{tc_i}_{oc}" + R)
                    for i in range(NT):
                        nc.tensor.matmul(ps_y[:],
                                         attnT[:, i, 128 * tc_i:128 * (tc_i + 1)],
                                         wo_sb[:, i, 512 * oc:512 * (oc + 1)],
                                         start=(i == 0), stop=(i == NT - 1))
                    osb = tmp.tile([128, 512], F32, tag="osb", name=f"osb{tc_i}_{oc}" + R)
                    nc.vector.tensor_copy(osb[:], ps_y[:])
                    nc.sync.dma_start(
                        y_d[128 * tc_i:128 * (tc_i + 1), 512 * oc:512 * (oc + 1)],
                        osb[:])


def _build(repeat=1):
    key = ("nc", repeat)
    if key in _CACHE:
        return _CACHE[key]
    nc = bacc.Bacc("TRN2", target_bir_lowering=False, debug=False,
                   enable_asserts=False, num_devices=NC)
    with tile.TileContext(nc) as tc:
        _emit(tc, repeat)
    nc.compile()
    nc.m = get_hw_module(nc.m)
    _CACHE[key] = nc
    return nc


# ---------------------------------------------------------------------------
# host-side input preparation
# ---------------------------------------------------------------------------

def _pos_table(tab, f, h, w):
    cf = C - 2 * (C // 3)
    ch = C // 3
    tf = np.broadcast_to(tab[:f, :cf][:, None, None, :], (f, h, w, cf))
    th = np.broadcast_to(tab[:h, cf:cf + ch][None, :, None, :], (f, h, w, ch))
    tw = np.broadcast_to(tab[:w, cf + ch:][None, None, :, :], (f, h, w, ch))
    return np.concatenate([tf, th, tw], axis=-1).reshape(f * h * w, C)


def _rope_tables(cosP, sinP, g, scale):
    """(cosT, sinT) [S, DIM] folding g and the score scale.

    Device computes: rot = pre*cosT + swap(pre)*sinT, where swap exchanges
    even/odd partners. Equivalent to scale * rope(g * pre)."""
    cosE = np.repeat(cosP, 2, axis=1)          # [S, HD]
    sinE = np.repeat(sinP, 2, axis=1)
    cosT = np.empty((S, DIM), np.float32)
    sinT = np.empty((S, DIM), np.float32)
    for n in range(NH):
        gh = g[128 * n:128 * (n + 1)]
        cosT[:, 128 * n:128 * (n + 1)] = cosE * gh[None, :] * scale
        sh = np.empty(HD, np.float32)
        sh[0::2] = -gh[1::2]
        sh[1::2] = gh[0::2]
        sinT[:, 128 * n:128 * (n + 1)] = sinE * sh[None, :] * scale
    return cosT, sinT


def _ctx_token_ids(c):
    wbase = 2 * max(c // 2 - 1, 0)
    return np.concatenate([np.arange(256),
                           np.arange(T * wbase, T * wbase + 1536)])


def _mask_for_core(c):
    qb = c // 2
    frame = np.arange(S) // 256
    blk = frame // NFPB

    def allowed(k):
        return (blk[k] <= qb) & (((qb - blk[k]) * NFPB < LOCAL) | (frame[k] < SINK))

    m = np.full(CTX, MASK_NEG, np.float32)
    if qb >= 2:
        m[0:256] = 0.0
    wbase = 2 * max(qb - 1, 0)
    tok = np.arange(T * wbase, T * wbase + 1536)
    m[256:] = np.where(allowed(tok), 0.0, MASK_NEG)
    return np.ascontiguousarray(m.reshape(NCT, 128).T)  # [128, NCT]


def _prep_in_maps(x, Wq, bq, Wk, bk, Wv, bv, Wo, bo, gq, gk, freqs_cos, freqs_sin, f, h, w):
    x = np.asarray(x, np.float32)
    f, h, w = int(f), int(h), int(w)
    cosP = _pos_table(np.asarray(freqs_cos, np.float32), f, h, w)
    sinP = _pos_table(np.asarray(freqs_sin, np.float32), f, h, w)

    qcosT, qsinT = _rope_tables(cosP, sinP, np.asarray(gq, np.float32), HD ** -0.5)
    kcosT, ksinT = _rope_tables(cosP, sinP, np.asarray(gk, np.float32), 1.0)

    import ml_dtypes
    BF = ml_dtypes.bfloat16
    wq_t = np.ascontiguousarray(np.asarray(Wq, np.float32).T).astype(BF)
    wk_t = np.ascontiguousarray(np.asarray(Wk, np.float32).T).astype(BF)
    wv_t = np.ascontiguousarray(np.asarray(Wv, np.float32).T).astype(BF)
    wo_t = np.ascontiguousarray(np.asarray(Wo, np.float32).T).astype(BF)
    bq_r = np.ascontiguousarray(np.asarray(bq, np.float32).reshape(NT, 128).T)
    bk_r = np.ascontiguousarray(np.asarray(bk, np.float32).reshape(NT, 128).T)

    swapm = np.zeros((128, 128), np.float32)
    idx = np.arange(128)
    swapm[idx, idx ^ 1] = 1.0
    swapm = swapm.astype(BF)
    onesc = np.ones((128, 1), BF)
    onesrow = np.ones((1, 128), np.float32)

    xs = x[0]  # [S, DIM]
    in_maps = []
    for c in range(NC):
        sl = slice(T * c, T * (c + 1))
        ids = _ctx_token_ids(c)
        in_maps.append(dict(
            xq=np.ascontiguousarray(xs[sl].T).astype(BF),
            xc=np.ascontiguousarray(xs[ids].T).astype(BF),
            wq=wq_t, wk=wk_t, wv=wv_t, wo=wo_t,
            bqr=bq_r, bkr=bk_r,
            qcos=np.ascontiguousarray(qcosT[sl].T).astype(BF),
            qsin=np.ascontiguousarray(qsinT[sl].T).astype(BF),
            kcos=np.ascontiguousarray(kcosT[ids].T).astype(BF),
            ksin=np.ascontiguousarray(ksinT[ids].T).astype(BF),
            swapm=swapm, onesc=onesc, onesrow=onesrow, maskb=_mask_for_core(c),
        ))

    bo_eff = np.asarray(bo, np.float32) + np.asarray(bv, np.float32) @ np.asarray(Wo, np.float32).T
    return in_maps, bo_eff


def _assemble(per_core_y, bo_eff):
    out = np.concatenate(per_core_y, axis=0)  # [S, DIM]
    out = out + bo_eff[None, :]
    return out[None].astype(np.float32)


# ---------------------------------------------------------------------------
# execution (PJRT shard_map; replicated specs for weights)
# ---------------------------------------------------------------------------

def _make_runner(nc):
    import jax
    from jax.sharding import Mesh, PartitionSpec
    try:
        from jax.experimental.shard_map import shard_map
    except ImportError:
        from jax.shard_map import shard_map
    from concourse.bass2jax import _bass_exec_p, install_neuronx_cc_hook, partition_id_tensor

    install_neuronx_cc_hook()
    partition_name = nc.partition_id_tensor.name if nc.partition_id_tensor else None
    in_names, out_names, out_avals = [], [], []
    for alloc in nc.m.functions[0].allocations:
        if not isinstance(alloc, mybir.MemoryLocationSet):
            continue
        name = alloc.memorylocations[0].name
        if alloc.kind == "ExternalInput":
            if name != partition_name:
                in_names.append(name)
        elif alloc.kind == "ExternalOutput":
            out_names.append(name)
            out_avals.append(jax.core.ShapedArray(tuple(alloc.tensor_shape),
                                                  mybir.dt.np(alloc.dtype)))
    n_params = len(in_names)
    all_in_names = list(in_names) + out_names
    if partition_name is not None:
        all_in_names.append(partition_name)

    def _body(*args):
        ins = list(args[:n_params])
        zouts = list(args[n_params:])
        extra = [partition_id_tensor()] if partition_name is not None else []
        outs = _bass_exec_p.bind(
            *ins, *zouts, *extra,
            out_avals=tuple(out_avals),
            in_names=tuple(all_in_names),
            out_names=tuple(out_names),
            lowering_input_output_aliases=(),
            sim_require_finite=False,
            sim_require_nnan=False,
            nc=nc,
        )
        return tuple(outs)

    import numpy as _np
    devices = jax.devices()[:NC]
    mesh = Mesh(_np.asarray(devices), ("core",))
    in_specs = tuple(
        PartitionSpec() if name in REPLICATED else PartitionSpec("core")
        for name in in_names
    ) + (PartitionSpec("core"),) * len(out_names)
    out_specs = (PartitionSpec("core"),) * len(out_names)
    fn = jax.jit(shard_map(_body, mesh=mesh, in_specs=in_specs,
                           out_specs=out_specs, check_rep=False))
    return fn, in_names, out_names, out_avals


def _prepare_args(in_maps, in_names, out_avals):
    import jax
    args = []
    for i, name in enumerate(in_names):
        if name in REPLICATED:
            args.append(in_maps[0][name])
        else:
            args.append(np.concatenate([np.asarray(m[name]) for m in in_maps], axis=0))
    for a in out_avals:
        args.append(np.zeros((NC * a.shape[0], *a.shape[1:]), a.dtype))
    return [jax.device_put(a) for a in args]


def _run(nc, in_maps):
    import jax
    key = ("runner", id(nc))
    if key not in _CACHE:
        _CACHE[key] = _make_runner(nc)
    fn, in_names, out_names, out_avals = _CACHE[key]
    args = _prepare_args(in_maps, in_names, out_avals)
    outs = fn(*args)
    jax.block_until_ready(outs)
    results = []
    for c in range(NC):
        r = {}
        for i, name in enumerate(out_names):
            r[name] = np.asarray(outs[i]).reshape(NC, *out_avals[i].shape)[c]
        results.append(r)
    return results


def kernel(**inputs):
    global LAST_RESULT
    in_maps, bo_eff = _prep_in_maps(**inputs)
    nc = _build()
    results = _run(nc, in_maps)
    LAST_RESULT = results
    return _assemble([results[c]["y"] for c in range(NC)], bo_eff)
